# revision 62
# baseline (speedup 1.0000x reference)
"""Trainium2 Bass kernel for BatchedDifferentiableDynamicBicycleModel.

Contract: kernel(state=[B,9] f32, action=[B,2] f32, dt=scalar) -> [B,9] f32.
B = 262144, sharded batch-parallel across 8 NeuronCores (32768 vehicles each,
one [128, 256] f32 tile per state variable).

dt=1 -> 100 fp32 Euler substeps (all h = f32(0.01)). v2 schedule: the
critical cycle is r/beta -> rv -> X/Y (PE diag matmuls, fp32r) -> tanh (ACT)
-> S1/rq (PE) -> w (DVE) -> beta'/r'. All fp32->fp32r matmul operands are
bitcast views (no mirror copies); beta - h*r is prestaged on GpSimd off the
cycle; abs(phi) runs on ACT; ha = h*a follows the exact closed-form decay
ha' = QA*ha + (h^2/tau)*aref (1 op/step). v, ha, delta, inv are parity
double-buffered so every op reads old state regardless of queue position.

Engine split per substep:
  PE:  psi += h*r, X(3 mm), Y(2 mm), S1(2 mm), rq(2 mm), xy += h*[vc|vs]
  ACT: sin(phi), abs(phi), cos=sin(pi/2-|phi|), tanh over strided [X|Y]
  DVE: rv=r*inv (->f32r), dclip, relu-v, recip (seeded NR2), vc=v*cos,
       w=S1*inv, beta'=bmh+w, phi'=wrap(phi+w)
  GpSimd: bmh=beta-h*r, ha'=QA*ha+haref2, vs=v*sin, r'=r+rq
"""

import math
import os
import sys

for _p in ("/opt/trn_rl_repo", "/opt/pypackages"):
    if _p not in sys.path:
        sys.path.insert(0, _p)

import numpy as np

# ----------------------------------------------------------------------------
# Model constants (match reference.py bit-for-bit in float64)
# ----------------------------------------------------------------------------
M_, IZ, LF, LR, CF, CR = 1500.0, 2250.0, 1.2, 1.6, 80000.0, 80000.0
TAU_A, TAU_D = 0.1, 0.1
MAX_STEER = 30.0 * np.pi / 180.0
MAX_ACC, MIN_ACC = 3.0, -6.0
MU, G = 0.9, 9.81
L = LF + LR
FY_F_MAX = MU * M_ * G * (LR / L)
FY_R_MAX = MU * M_ * G * (LF / L)
DT_INTERNAL = 0.01
V_EFF_MIN = 20.0 / 3.6

N_CORES = 8
B_TOTAL = 262144
B_CORE = B_TOTAL // N_CORES  # 32768
P = 128

_f32 = np.float32

# ----------------------------------------------------------------------------
# Custom DVE ops
# ----------------------------------------------------------------------------
_REG = {}


def _register_custom_ops():
    import concourse.dve_ops as dom
    from concourse.dve_ops import DveOp
    from concourse.dve_spec import (
        Spec, Src0, Src1, C0, C1, C2, lower, maxx, minn, relu, _has_src1,
    )
    from concourse.dve_uop import DveOpSpec

    def reg(name, spec):
        if name in dom._SUB_OPCODE_FOR_NAME:
            _REG[name] = next(op for op in dom.OPS if op.name == name)
            return
        opcode = dom._CUSTOM_DVE_ROW_BASE + len(dom.OPS)
        assert opcode < 0x20, "custom DVE row overflow"
        dom._SUB_OPCODE_FOR_NAME[name] = opcode
        shas = {}
        for ver in ("v3", "v4"):
            s = DveOpSpec(name=name, opcode=opcode, uops=lower(spec, ver=ver),
                          rd1_en=_has_src1(spec))
            shas[ver] = s.sha(ver)
        op = DveOp(name, spec, subdim=False, uops_sha=shas)
        dom.OPS.append(op)
        dom.CUSTOM_DVE_SPECS[name] = spec
        _REG[name] = op

    # inv' = NR2(max(v, s0), seed=in1); s1 = 2.0
    def _recip_ref(in0, in1, s0, s1, imm2):
        ve = np.maximum(in0, s0).astype(np.float32)
        y1 = (in1 * (s1 - ve * in1)).astype(np.float32)
        return (y1 * (s1 - ve * y1)).astype(np.float32)

    _ve = maxx(Src0, C0)
    _y1 = Src1 * (C1 - _ve * Src1)
    reg("ANT_BIKE_RECIP_NR2", Spec(body=_y1 * (C1 - _ve * _y1),
                                   reference=_recip_ref))

    # phi' = wrap_pm_pi(phi + w): y = in0+in1; y + imm2*((y<-s0)-(y>s0))
    def _phistep_ref(in0, in1, s0, s1, imm2):
        y = (in0 + in1).astype(np.float32)
        lo = (y < -s0).astype(np.float32)
        hi = (y > s0).astype(np.float32)
        return (y + imm2 * (lo - hi)).astype(np.float32)

    _y = Src0 + Src1
    reg("ANT_BIKE_PHI_STEP", Spec(body=_y + C2 * ((_y < -C0) - (_y > C0)),
                                  reference=_phistep_ref))

    # full wrap to [-pi,pi]: k = rn(x*s0) via magic s1; out = x - k*imm2
    def _wraprn_ref(in0, in1, s0, s1, imm2):
        t = (in0 * s0).astype(np.float32)
        k = ((t + s1).astype(np.float32) - s1).astype(np.float32)
        return (in0 - k * imm2).astype(np.float32)

    _k = (Src0 * C0 + C1) - C1
    reg("ANT_BIKE_WRAP_RN", Spec(body=Src0 - _k * C2, reference=_wraprn_ref))

    # delta' = clip(delta*s0 + dref*imm2, s1, -s1)  (s1 = -MAX_STEER)
    def _dclip_ref(in0, in1, s0, s1, imm2):
        z = (in0 * s0 + in1 * imm2).astype(np.float32)
        return np.minimum(np.maximum(z, s1), -np.float32(s1)).astype(np.float32)

    _z = Src0 * C0 + Src1 * C2
    reg("ANT_BIKE_DCLIP", Spec(body=minn(maxx(_z, C1), -C1),
                               reference=_dclip_ref))

    # v' = relu(in0 + in1*s0)
    def _reluadd_ref(in0, in1, s0, s1, imm2):
        z = (in0 + in1 * s0).astype(np.float32)
        return np.maximum(np.nan_to_num(z, nan=0.0, posinf=np.inf,
                                        neginf=-np.inf), 0).astype(np.float32)

    reg("ANT_BIKE_RELUADD", Spec(body=relu(Src0 + Src1 * C0),
                                 reference=_reluadd_ref))

    # inv' = NR1(max(v, s0), seed=in1); s1 = 2.0 -- one Newton step off the
    # previous step's inv (v moves <= 1.1% per step, so err <= ~1.2e-4)
    def _recip1_ref(in0, in1, s0, s1, imm2):
        ve = np.maximum(in0, s0).astype(np.float32)
        return (in1 * (s1 - ve * in1)).astype(np.float32)

    _ve1 = maxx(Src0, C0)
    reg("ANT_BIKE_RECIP_NR1", Spec(body=Src1 * (C1 - _ve1 * Src1),
                                   reference=_recip1_ref))


# ----------------------------------------------------------------------------
# Kernel builder
# ----------------------------------------------------------------------------

def _step_hs(dt_total):
    """Replicate the reference's python-float substep splitting."""
    n_full = int(dt_total // DT_INTERNAL)
    dt_rem = dt_total - n_full * DT_INTERNAL
    hs = [DT_INTERNAL] * n_full
    if dt_rem > 0.0:
        hs.append(dt_rem)
    return hs


def build_kernel(hs, n_veh=B_CORE):
    _register_custom_ops()
    import concourse.bacc as bacc
    import concourse.bass as bass
    import concourse.tile as tile
    from concourse import mybir
    from concourse.mybir import AluOpType as alu
    ACT = mybir.ActivationFunctionType

    FD = n_veh // P
    n_steps = len(hs)

    hs32 = [_f32(h) for h in hs]
    h_base = float(hs32[0]) if n_steps else DT_INTERNAL
    MS = _f32(MAX_STEER)
    VMIN = _f32(V_EFF_MIN)
    CFS = _f32(-CF / FY_F_MAX)
    CRS = _f32(-CR / FY_R_MAX)
    PI_F = _f32(np.pi)
    TWO_PI = _f32(2.0 * np.pi)
    INV_2PI = _f32(1.0 / (2.0 * np.pi))
    MAGIC = _f32(12582912.0)
    HALF_PI = _f32(np.pi / 2.0)

    # diag weights: [1, cf, -cf, cf*LF, cr, -cr*LR] + per h: [h, c1, c2, k1, k2]
    cfd = float(CFS)
    crd = float(CRS)
    shared = [1.0, cfd, -cfd, float(_f32(cfd * LF)), crd,
              float(_f32(-crd * LR)), -1.0]
    hdiag = {}
    dset = list(shared)
    for h32 in sorted(set(float(v) for v in hs32)):
        h = float(h32)
        vals = [h,
                float(_f32(h * FY_F_MAX / M_)), float(_f32(h * FY_R_MAX / M_)),
                float(_f32(h * LF * FY_F_MAX / IZ)),
                float(_f32(-h * LR * FY_R_MAX / IZ))]
        hdiag[h] = list(range(len(dset), len(dset) + len(vals)))
        dset.extend(vals)
    D_ONE, D_CF, D_NCF, D_CFLF, D_CR, D_NCRLR, D_NEG1 = 0, 1, 2, 3, 4, 5, 6
    ND = len(dset)

    wdiag_host = np.zeros((ND, P, P), dtype=np.float32)
    eye = np.eye(P, dtype=np.float32)
    for i, c in enumerate(dset):
        wdiag_host[i] = eye * _f32(c)

    nc = bacc.Bacc("TRN2", target_bir_lowering=False, debug=False)
    st_d = nc.declare_dram_parameter("state", [n_veh, 9], mybir.dt.float32,
                                     isOutput=False)
    ac_d = nc.declare_dram_parameter("action", [n_veh, 2], mybir.dt.float32,
                                     isOutput=False)
    wd_d = nc.declare_dram_parameter("wdiag", [ND, P, P], mybir.dt.float32,
                                     isOutput=False)
    out_d = nc.declare_dram_parameter("out", [n_veh, 9], mybir.dt.float32,
                                      isOutput=True)

    f32 = mybir.dt.float32
    f32r = mybir.dt.float32r

    RECIP = _REG["ANT_BIKE_RECIP_NR1"]
    PHISTEP = _REG["ANT_BIKE_PHI_STEP"]
    WRAPRN = _REG["ANT_BIKE_WRAP_RN"]
    DCLIP = _REG["ANT_BIKE_DCLIP"]
    RELUADD = _REG["ANT_BIKE_RELUADD"]

    with tile.TileContext(nc) as tc:
        with (
            tc.tile_pool(name="persist", bufs=1) as pp,
            tc.tile_pool(name="scratch", bufs=2) as sp,
            tc.tile_pool(name="psum", bufs=1, space="PSUM") as qq,
        ):
            # persistent SBUF
            big_in = pp.tile([P, FD * 9], f32)
            big_ac = pp.tile([P, FD * 2], f32)
            big_out = pp.tile([P, FD * 9], f32)
            wsb = pp.tile([P, ND * P], f32)
            wsr = pp.tile([P, ND * P], f32r)
            # beta lives in PSUM: beta_q += mm(-1,hr) + mm(1,w) per step --
            # PSUM adds are exact fp32 and only the f32r-rounded INCREMENTS
            # carry rounding (~1e-5 total). The X/Y matmuls read an f32r
            # SBUF mirror refreshed per step by ACT (PSUM reads are cheap
            # there). This keeps GpSimd nearly idle: DVE and GpSimd share
            # SBUF ports, and concurrent GpSimd traffic was measured to
            # inflate every DVE op 2-3x.
            beta_m = pp.tile([P, FD], f32r)
            # parity double-buffers: step k reads [k%2], writes [(k+1)%2];
            # lets updates run early in the queue while late readers still
            # see the old value.
            v_b = [pp.tile([P, FD], f32, name=f"v{i}") for i in range(2)]
            inv_b = [pp.tile([P, FD], f32, name=f"inv{i}") for i in range(2)]
            phi_b = [pp.tile([P, FD], f32, name=f"phi{i}") for i in range(2)]
            # delta: the clip can only bind on step 0 (afterwards |delta|
            # stays <= MAX_STEER and the recurrence is affine), so steps
            # k>=1 use delta_k = dref + dd with dd (f32r) decaying by QD.
            de0 = pp.tile([P, FD], f32)
            de1 = pp.tile([P, FD], f32)
            dd_t = pp.tile([P, FD], f32r)
            dref_m = pp.tile([P, FD], f32r)
            va_t = pp.tile([P, FD], f32)       # h*a (exact stt recurrence)
            haref2_c = pp.tile([P, FD], f32)   # (h^2/tau_a)*aref
            dref_c = pp.tile([P, FD], f32)
            qd_full = pp.tile([P, FD], f32)
            halfpi_b = pp.tile([P, 1], f32)
            nc.gpsimd.memset(halfpi_b[:], float(HALF_PI))
            # Pin the ACT table set: Silu exists only in silu_and_others,
            # which also holds Sin/Tanh/Abs/Copy -- one table load total.
            nc.scalar.activation(halfpi_b[:], halfpi_b[:], ACT.Silu)
            nc.gpsimd.memset(halfpi_b[:], float(HALF_PI))

            # PSUM: xy 1 bank, XY 2 (X in bank A, Y in bank B), S1 1;
            # r and psi are persistent PSUM accumulators (init hi/lo mms,
            # then two/one accumulating mms per step; V reads them mid-group)
            xy_q = qq.tile([P, 2 * FD], f32)
            XY_q = qq.tile([P, 4 * FD], f32)
            X_q = XY_q[:, 0:FD]
            Y_q = XY_q[:, 2 * FD:3 * FD]
            S1_q = qq.tile([P, FD], f32)
            r_q = qq.tile([P, FD], f32)
            psi_q = qq.tile([P, FD], f32)
            beta_q = qq.tile([P, FD], f32)

            def W(i):
                return wsr[:, bass.ts(i, P)]

            def mm(out_ap, didx, rhs_ap, start, stop):
                nc.tensor.matmul(out_ap, W(didx), rhs_ap,
                                 start=start, stop=stop)

            # ---------------- load + unpack ----------------
            nc.sync.dma_start(big_in[:], st_d[:].rearrange(
                "(p q) v -> p (q v)", p=P))
            nc.sync.dma_start(big_ac[:], ac_d[:].rearrange(
                "(p q) v -> p (q v)", p=P))
            nc.sync.dma_start(wsb[:].rearrange("p (d m) -> p d m", m=P),
                              wd_d[:].rearrange("d k m -> k d m"))
            nc.vector.tensor_copy(wsr[:], wsb[:])  # fp32 -> fp32r weights

            sv = big_in[:].rearrange("p (q v) -> p q v", v=9)
            av = big_ac[:].rearrange("p (q v) -> p q v", v=2)
            xy0 = sp.tile([P, 2 * FD], f32, tag="init0")
            psi0 = sp.tile([P, FD], f32, tag="init2")
            a0 = sp.tile([P, FD], f32, tag="init3")
            r0 = sp.tile([P, FD], f32, tag="init_r")
            beta0 = sp.tile([P, FD], f32, tag="init_beta")
            nc.vector.tensor_copy(xy0[:, 0:FD], sv[:, :, 0])
            nc.vector.tensor_copy(xy0[:, FD:2 * FD], sv[:, :, 1])
            nc.vector.tensor_copy(psi0[:], sv[:, :, 2])
            nc.scalar.copy(v_b[0][:], sv[:, :, 3])
            nc.scalar.copy(a0[:], sv[:, :, 4])
            nc.vector.tensor_copy(de0[:], sv[:, :, 5])
            nc.vector.tensor_copy(beta0[:], sv[:, :, 6])
            nc.vector.tensor_copy(r0[:], sv[:, :, 7])

            aref_c = sp.tile([P, FD], f32, tag="init_aref")
            nc.vector.tensor_scalar(aref_c[:], av[:, :, 0], float(MIN_ACC),
                                    float(MAX_ACC), alu.max, alu.min)
            nc.vector.tensor_scalar(dref_c[:], av[:, :, 1], float(-MS),
                                    float(MS), alu.max, alu.min)
            nc.scalar.copy(dref_m[:], dref_c[:])
            nc.gpsimd.memset(qd_full[:], float(_f32(1.0)
                                               - _f32(_f32(h_base)
                                                      / _f32(TAU_D))))
            # va = h*a ; haref2 = (h^2/tau_a)*aref
            nc.vector.tensor_scalar(va_t[:], a0[:], h_base, None, alu.mult)
            nc.vector.tensor_scalar(haref2_c[:], aref_c[:],
                                    float(_f32(_f32(_f32(h_base)
                                               * _f32(h_base))
                                               / _f32(TAU_A))),
                                    None, alu.mult)
            # exact reciprocal seed
            ve0 = sp.tile([P, FD], f32, tag="init4")
            nc.vector.tensor_scalar(ve0[:], v_b[0][:], float(VMIN), None,
                                    alu.max)
            nc.vector.reciprocal(inv_b[0][:], ve0[:])
            # wrapped phi = psi + beta
            pb0 = sp.tile([P, FD], f32, tag="init5")
            nc.vector.tensor_add(pb0[:], psi0[:], beta0[:])
            nc.vector._custom_dve(WRAPRN, out=phi_b[0][:], in0=pb0[:],
                                  s0=float(INV_2PI), s1=float(MAGIC),
                                  imm2=float(TWO_PI))

            # exact PSUM init via hi/lo split (fp32r rounds moving values)
            xy0_hi = sp.tile([P, 2 * FD], f32r, tag="init6")
            xy0_lo = sp.tile([P, 2 * FD], f32r, tag="init7")
            psi0_hi = sp.tile([P, FD], f32r, tag="init8")
            psi0_lo = sp.tile([P, FD], f32r, tag="init9")
            r0_hi = sp.tile([P, FD], f32r, tag="init10")
            r0_lo = sp.tile([P, FD], f32r, tag="init11")
            b0_hi = sp.tile([P, FD], f32r, tag="init12")
            b0_lo = sp.tile([P, FD], f32r, tag="init13")
            nc.scalar.copy(xy0_hi[:], xy0[:])
            nc.vector.tensor_tensor(xy0_lo[:], xy0[:], xy0_hi[:],
                                    alu.subtract)
            nc.scalar.copy(psi0_hi[:], psi0[:])
            nc.vector.tensor_tensor(psi0_lo[:], psi0[:], psi0_hi[:],
                                    alu.subtract)
            nc.scalar.copy(r0_hi[:], r0[:])
            nc.vector.tensor_tensor(r0_lo[:], r0[:], r0_hi[:], alu.subtract)
            nc.scalar.copy(b0_hi[:], beta0[:])
            nc.vector.tensor_tensor(b0_lo[:], beta0[:], b0_hi[:],
                                    alu.subtract)
            nc.scalar.copy(beta_m[:], beta0[:])
            laststop = n_steps == 0
            mm(xy_q[:], D_ONE, xy0_hi[:], start=True, stop=False)
            mm(xy_q[:], D_ONE, xy0_lo[:], start=False, stop=laststop)
            mm(psi_q[:], D_ONE, psi0_hi[:], start=True, stop=False)
            mm(psi_q[:], D_ONE, psi0_lo[:], start=False, stop=laststop)
            mm(r_q[:], D_ONE, r0_hi[:], start=True, stop=False)
            mm(r_q[:], D_ONE, r0_lo[:], start=False, stop=laststop)
            mm(beta_q[:], D_ONE, b0_hi[:], start=True, stop=False)
            mm(beta_q[:], D_ONE, b0_lo[:], start=False, stop=laststop)

            # ---------------- main loop ----------------
            # Software-pipelined: rv for step k is computed at the tail of
            # step k-1 (right after w, when r is fresh); the xy matmul for
            # step k-1's trig runs at the START of step k, filling the PE
            # gap and keeping it warm. Both carry tiles across iterations.
            rv_cur = sp.tile([P, FD], f32r, tag="rv")
            nc.vector.tensor_tensor(rv_cur[:], r_q[:], inv_b[0][:], alu.mult)
            vcvs_prev = None
            for k in range(n_steps):
                h = float(hs32[k])
                DH, DC1, DC2, DK1, DK2 = hdiag[h]
                QA = float(_f32(1.0) - _f32(_f32(h) / _f32(TAU_A)))
                QD = float(_f32(1.0) - _f32(_f32(h) / _f32(TAU_D)))
                CD = float(_f32(_f32(h) / _f32(TAU_D)))
                hh = float(_f32(h) / _f32(h_base))
                last = (k + 1 == n_steps)

                v_cur, v_nxt = v_b[k % 2], v_b[(k + 1) % 2]
                inv_cur, inv_nxt = inv_b[k % 2], inv_b[(k + 1) % 2]
                phi_cur, phi_nxt = phi_b[k % 2], phi_b[(k + 1) % 2]

                # scratch tiles for this step (double-buffered pool)
                hr = sp.tile([P, FD], f32r, tag="hr")
                trig = sp.tile([P, 2 * FD], f32, tag="trig")
                absphi = sp.tile([P, FD], f32, tag="absphi")
                TfTr = sp.tile([P, 2 * FD], f32r, tag="TfTr")
                Tf = TfTr[:, 0:FD]
                Tr = TfTr[:, FD:2 * FD]
                w_t = sp.tile([P, FD], f32r, tag="w")
                vcvs = sp.tile([P, 2 * FD], f32r, tag="vcvs")

                # ---- ACT head: hr = h*r (PSUM read, f32r out) ----
                nc.scalar.mul(hr[:], r_q[:], h)
                if k == 0:
                    # step 0 uses the raw (possibly out-of-range) delta
                    de_m = sp.tile([P, FD], f32r, tag="de_m")
                    nc.scalar.copy(de_m[:], de0[:])

                # ---- PE: previous step's xy accumulation first -- fills
                # the cold gap and warms the array for the X chain ----
                if vcvs_prev is not None:
                    mm(xy_q[:], DH_prev, vcvs_prev[:], start=False,
                       stop=False)

                # ---- PE: X, Y. X delta terms go first (constant operands,
                # run before rv lands); beta mms last (freshest dep) ----
                if k == 0:
                    mm(X_q, D_NCF, de_m[:], start=True, stop=False)
                else:
                    mm(X_q, D_NCF, dref_m[:], start=True, stop=False)
                    mm(X_q, D_NCF, dd_t[:], start=False, stop=False)
                mm(X_q, D_CFLF, rv_cur[:], start=False, stop=False)
                mm(Y_q, D_NCRLR, rv_cur[:], start=True, stop=False)
                mm(X_q, D_CF, beta_m[:], start=False, stop=True)
                mm(Y_q, D_CR, beta_m[:], start=False, stop=True)

                # ---- DVE fillers (run while PE/ACT chew on X/tanh) ----
                if k == 0:
                    nc.vector._custom_dve(DCLIP, out=de1[:], in0=de0[:],
                                          in1=dref_c[:], s0=QD,
                                          s1=float(-MS), imm2=CD)
                    if n_steps > 1:
                        # dd = delta_1 - dref (f32r; decays by QD onwards)
                        nc.vector.tensor_tensor(dd_t[:], de1[:], dref_c[:],
                                                alu.subtract)
                nc.vector._custom_dve(RELUADD, out=v_nxt[:], in0=v_cur[:],
                                      in1=va_t[:], s0=hh)
                if not last:
                    nc.vector._custom_dve(RECIP, out=inv_nxt[:],
                                          in0=v_nxt[:], in1=inv_cur[:],
                                          s0=float(VMIN), s1=2.0)
                # va' = QA*va + (h^2/tau)*aref (reads old va after RELUADD)
                nc.vector.scalar_tensor_tensor(va_t[:], va_t[:], QA,
                                               haref2_c[:], alu.mult, alu.add)

                # one tanh over [X | Y] (strided 2-bank PSUM read)
                nc.scalar.activation(
                    TfTr[:].rearrange("p (a b) -> p a b", a=2),
                    XY_q[:].rearrange("p (a b) -> p a b", a=4)[:, 0::2, :],
                    ACT.Tanh)

                # psi += h*r and beta -= h*r here: fills the PE idle window
                # under tanh so the S1 matmul issues warm
                mm(psi_q[:], D_ONE, hr[:], start=False, stop=last)
                mm(beta_q[:], D_NEG1, hr[:], start=False, stop=False)

                # S1 = c1*Tf + c2*Tr ; r += k1*Tf + k2*Tr (open accumulator)
                mm(S1_q[:], DC1, Tf, start=True, stop=False)
                mm(S1_q[:], DC2, Tr, start=False, stop=True)
                mm(r_q[:], DK1, Tf, start=False, stop=False)
                mm(r_q[:], DK2, Tr, start=False, stop=last)

                # ---- cycle tail: w (f32r) -> beta_q += w on PE ----
                nc.vector.tensor_tensor(w_t[:], S1_q[:], inv_cur[:],
                                        alu.mult)
                mm(beta_q[:], D_ONE, w_t[:], start=False, stop=last)
                # f32r beta mirror for next step's X/Y matmuls (ACT reads
                # PSUM cheaply; DVE ts pays ~740ns for the f32r convert)
                nc.scalar.copy(beta_m[:], beta_q[:])
                if not last:
                    # rv for the NEXT step: r_q now holds r_{k+1}
                    rv_nxt = sp.tile([P, FD], f32r, tag="rv")
                    nc.vector.tensor_tensor(rv_nxt[:], r_q[:], inv_nxt[:],
                                            alu.mult)
                    nc.vector._custom_dve(PHISTEP, out=phi_nxt[:],
                                          in0=phi_cur[:],
                                          in1=w_t[:], s0=float(PI_F),
                                          imm2=float(TWO_PI))
                    if k >= 1:
                        # decay dd for the next step (ACT copy-with-scale;
                        # GpSimd traffic inflates concurrent DVE ops)
                        nc.scalar.mul(dd_t[:], dd_t[:], QD)

                # ---- trig of phi_k for THIS step's xy increment (consumed
                # by the xy matmul at the start of step k+1; these read the
                # parity buffer so queue position is free) ----
                nc.scalar.activation(trig[:, FD:2 * FD], phi_cur[:], ACT.Sin)
                nc.scalar.activation(absphi[:], phi_cur[:], ACT.Abs)
                nc.scalar.activation(trig[:, 0:FD], absphi[:], ACT.Sin,
                                     bias=halfpi_b[:], scale=-1.0)
                nc.gpsimd.tensor_tensor(vcvs[:, FD:2 * FD],
                                        trig[:, FD:2 * FD],
                                        v_cur[:], alu.mult)
                nc.vector.tensor_tensor(vcvs[:, 0:FD], trig[:, 0:FD],
                                        v_cur[:], alu.mult)

                if not last:
                    rv_cur = rv_nxt
                vcvs_prev = vcvs
                DH_prev = DH

            # final step's xy accumulation
            if n_steps:
                mm(xy_q[:], DH_prev, vcvs_prev[:], start=False, stop=True)

            # ---------------- finalize ----------------
            pf = n_steps % 2
            ov = big_out[:].rearrange("p (q v) -> p q v", v=9)
            nc.vector.tensor_copy(ov[:, :, 0], xy_q[:, 0:FD])
            nc.vector.tensor_copy(ov[:, :, 1], xy_q[:, FD:2 * FD])
            nc.scalar.copy(ov[:, :, 2], psi_q[:])
            nc.vector.tensor_copy(ov[:, :, 3], v_b[pf][:])
            inv_hb = float(_f32(1.0) / _f32(h_base))
            nc.vector.tensor_scalar(ov[:, :, 4], va_t[:], inv_hb, None,
                                    alu.mult)
            if n_steps == 0:
                nc.vector.tensor_copy(ov[:, :, 5], de0[:])
            elif n_steps == 1:
                nc.vector.tensor_copy(ov[:, :, 5], de1[:])
            else:
                # delta_n = dref + QD_last * dd (one final decay)
                QDL = float(_f32(1.0)
                            - _f32(_f32(hs32[-1]) / _f32(TAU_D)))
                nc.vector.scalar_tensor_tensor(ov[:, :, 5], dd_t[:], QDL,
                                               dref_c[:], alu.mult, alu.add)
            nc.scalar.copy(ov[:, :, 6], beta_q[:])
            nc.vector.tensor_copy(ov[:, :, 7], r_q[:])
            nc.scalar.copy(ov[:, :, 8], dref_c[:])
            nc.sync.dma_start(out_d[:].rearrange("(p q) v -> p (q v)", p=P),
                              big_out[:])

    nc.compile()
    return nc, wdiag_host


_BUILD_CACHE = {}


def _get_built(dt_total, n_veh=B_CORE):
    hs = tuple(_step_hs(float(dt_total)))
    key = (hs, n_veh)
    if key not in _BUILD_CACHE:
        _BUILD_CACHE[key] = build_kernel(list(hs), n_veh)
    return _BUILD_CACHE[key]


def kernel(state, action, dt):
    state = np.ascontiguousarray(np.asarray(state, dtype=np.float32))
    action = np.ascontiguousarray(np.asarray(action, dtype=np.float32))
    assert state.shape == (B_TOTAL, 9) and action.shape == (B_TOTAL, 2)

    nc, wdiag = _get_built(float(dt))

    from concourse.bass_utils import run_bass_kernel_spmd

    st_sh = np.split(state, N_CORES, axis=0)
    ac_sh = np.split(action, N_CORES, axis=0)
    in_maps = [
        {"state": np.ascontiguousarray(st_sh[i]),
         "action": np.ascontiguousarray(ac_sh[i]),
         "wdiag": wdiag}
        for i in range(N_CORES)
    ]
    res = run_bass_kernel_spmd(nc, in_maps, core_ids=list(range(N_CORES)))
    out = np.concatenate([r["out"] for r in res.results], axis=0)
    return out.astype(np.float32)


if __name__ == "__main__":
    rng = np.random.default_rng(0)
    s = rng.standard_normal((B_TOTAL, 9), dtype=np.float32)
    a = rng.standard_normal((B_TOTAL, 2), dtype=np.float32)
    o = kernel(s, a, 1)
    print("out", o.shape, o.dtype, np.isfinite(o).all())


# revision 63
# speedup vs baseline: 1.1362x; 1.1362x over previous
"""Trainium2 Bass kernel for BatchedDifferentiableDynamicBicycleModel.

Contract: kernel(state=[B,9] f32, action=[B,2] f32, dt=scalar) -> [B,9] f32.
B = 262144, sharded batch-parallel across 8 NeuronCores (32768 vehicles each,
one [128, 256] f32 tile per state variable).

dt=1 -> 100 fp32 Euler substeps (all h = f32(0.01)). v2 schedule: the
critical cycle is r/beta -> rv -> X/Y (PE diag matmuls, fp32r) -> tanh (ACT)
-> S1/rq (PE) -> w (DVE) -> beta'/r'. All fp32->fp32r matmul operands are
bitcast views (no mirror copies); beta - h*r is prestaged on GpSimd off the
cycle; abs(phi) runs on ACT; ha = h*a follows the exact closed-form decay
ha' = QA*ha + (h^2/tau)*aref (1 op/step). v, ha, delta, inv are parity
double-buffered so every op reads old state regardless of queue position.

Engine split per substep:
  PE:  psi += h*r, X(3 mm), Y(2 mm), S1(2 mm), rq(2 mm), xy += h*[vc|vs]
  ACT: sin(phi), abs(phi), cos=sin(pi/2-|phi|), tanh over strided [X|Y]
  DVE: rv=r*inv (->f32r), dclip, relu-v, recip (seeded NR2), vc=v*cos,
       w=S1*inv, beta'=bmh+w, phi'=wrap(phi+w)
  GpSimd: bmh=beta-h*r, ha'=QA*ha+haref2, vs=v*sin, r'=r+rq
"""

import math
import os
import sys

for _p in ("/opt/trn_rl_repo", "/opt/pypackages"):
    if _p not in sys.path:
        sys.path.insert(0, _p)

import numpy as np

# ----------------------------------------------------------------------------
# Model constants (match reference.py bit-for-bit in float64)
# ----------------------------------------------------------------------------
M_, IZ, LF, LR, CF, CR = 1500.0, 2250.0, 1.2, 1.6, 80000.0, 80000.0
TAU_A, TAU_D = 0.1, 0.1
MAX_STEER = 30.0 * np.pi / 180.0
MAX_ACC, MIN_ACC = 3.0, -6.0
MU, G = 0.9, 9.81
L = LF + LR
FY_F_MAX = MU * M_ * G * (LR / L)
FY_R_MAX = MU * M_ * G * (LF / L)
DT_INTERNAL = 0.01
V_EFF_MIN = 20.0 / 3.6

N_CORES = 8
B_TOTAL = 262144
B_CORE = B_TOTAL // N_CORES  # 32768
P = 128

_f32 = np.float32

# ----------------------------------------------------------------------------
# Custom DVE ops
# ----------------------------------------------------------------------------
_REG = {}


def _register_custom_ops():
    import concourse.dve_ops as dom
    from concourse.dve_ops import DveOp
    from concourse.dve_spec import (
        Spec, Src0, Src1, C0, C1, C2, lower, maxx, minn, relu, _has_src1,
    )
    from concourse.dve_uop import DveOpSpec

    def reg(name, spec):
        if name in dom._SUB_OPCODE_FOR_NAME:
            _REG[name] = next(op for op in dom.OPS if op.name == name)
            return
        opcode = dom._CUSTOM_DVE_ROW_BASE + len(dom.OPS)
        assert opcode < 0x20, "custom DVE row overflow"
        dom._SUB_OPCODE_FOR_NAME[name] = opcode
        shas = {}
        for ver in ("v3", "v4"):
            s = DveOpSpec(name=name, opcode=opcode, uops=lower(spec, ver=ver),
                          rd1_en=_has_src1(spec))
            shas[ver] = s.sha(ver)
        op = DveOp(name, spec, subdim=False, uops_sha=shas)
        dom.OPS.append(op)
        dom.CUSTOM_DVE_SPECS[name] = spec
        _REG[name] = op

    # inv' = NR2(max(v, s0), seed=in1); s1 = 2.0
    def _recip_ref(in0, in1, s0, s1, imm2):
        ve = np.maximum(in0, s0).astype(np.float32)
        y1 = (in1 * (s1 - ve * in1)).astype(np.float32)
        return (y1 * (s1 - ve * y1)).astype(np.float32)

    _ve = maxx(Src0, C0)
    _y1 = Src1 * (C1 - _ve * Src1)
    reg("ANT_BIKE_RECIP_NR2", Spec(body=_y1 * (C1 - _ve * _y1),
                                   reference=_recip_ref))

    # phi' = wrap_pm_pi(phi + w): y = in0+in1; y + imm2*((y<-s0)-(y>s0))
    def _phistep_ref(in0, in1, s0, s1, imm2):
        y = (in0 + in1).astype(np.float32)
        lo = (y < -s0).astype(np.float32)
        hi = (y > s0).astype(np.float32)
        return (y + imm2 * (lo - hi)).astype(np.float32)

    _y = Src0 + Src1
    reg("ANT_BIKE_PHI_STEP", Spec(body=_y + C2 * ((_y < -C0) - (_y > C0)),
                                  reference=_phistep_ref))

    # full wrap to [-pi,pi]: k = rn(x*s0) via magic s1; out = x - k*imm2
    def _wraprn_ref(in0, in1, s0, s1, imm2):
        t = (in0 * s0).astype(np.float32)
        k = ((t + s1).astype(np.float32) - s1).astype(np.float32)
        return (in0 - k * imm2).astype(np.float32)

    _k = (Src0 * C0 + C1) - C1
    reg("ANT_BIKE_WRAP_RN", Spec(body=Src0 - _k * C2, reference=_wraprn_ref))

    # delta' = clip(delta*s0 + dref*imm2, s1, -s1)  (s1 = -MAX_STEER)
    def _dclip_ref(in0, in1, s0, s1, imm2):
        z = (in0 * s0 + in1 * imm2).astype(np.float32)
        return np.minimum(np.maximum(z, s1), -np.float32(s1)).astype(np.float32)

    _z = Src0 * C0 + Src1 * C2
    reg("ANT_BIKE_DCLIP", Spec(body=minn(maxx(_z, C1), -C1),
                               reference=_dclip_ref))

    # v' = relu(in0 + in1*s0)
    def _reluadd_ref(in0, in1, s0, s1, imm2):
        z = (in0 + in1 * s0).astype(np.float32)
        return np.maximum(np.nan_to_num(z, nan=0.0, posinf=np.inf,
                                        neginf=-np.inf), 0).astype(np.float32)

    reg("ANT_BIKE_RELUADD", Spec(body=relu(Src0 + Src1 * C0),
                                 reference=_reluadd_ref))

    # inv' = NR1(max(v, s0), seed=in1); s1 = 2.0 -- one Newton step off the
    # previous step's inv (v moves <= 1.1% per step, so err <= ~1.2e-4)
    def _recip1_ref(in0, in1, s0, s1, imm2):
        ve = np.maximum(in0, s0).astype(np.float32)
        return (in1 * (s1 - ve * in1)).astype(np.float32)

    _ve1 = maxx(Src0, C0)
    reg("ANT_BIKE_RECIP_NR1", Spec(body=Src1 * (C1 - _ve1 * Src1),
                                   reference=_recip1_ref))


# ----------------------------------------------------------------------------
# Kernel builder
# ----------------------------------------------------------------------------

def _step_hs(dt_total):
    """Replicate the reference's python-float substep splitting."""
    n_full = int(dt_total // DT_INTERNAL)
    dt_rem = dt_total - n_full * DT_INTERNAL
    hs = [DT_INTERNAL] * n_full
    if dt_rem > 0.0:
        hs.append(dt_rem)
    return hs


def build_kernel(hs, n_veh=B_CORE):
    _register_custom_ops()
    import concourse.bacc as bacc
    import concourse.bass as bass
    import concourse.tile as tile
    from concourse import mybir
    from concourse.mybir import AluOpType as alu
    ACT = mybir.ActivationFunctionType

    FD = n_veh // P
    n_steps = len(hs)

    hs32 = [_f32(h) for h in hs]
    h_base = float(hs32[0]) if n_steps else DT_INTERNAL
    MS = _f32(MAX_STEER)
    VMIN = _f32(V_EFF_MIN)
    CFS = _f32(-CF / FY_F_MAX)
    CRS = _f32(-CR / FY_R_MAX)
    PI_F = _f32(np.pi)
    TWO_PI = _f32(2.0 * np.pi)
    INV_2PI = _f32(1.0 / (2.0 * np.pi))
    MAGIC = _f32(12582912.0)
    HALF_PI = _f32(np.pi / 2.0)

    # diag weights: [1, cf, -cf, cf*LF, cr, -cr*LR] + per h: [h, c1, c2, k1, k2]
    cfd = float(CFS)
    crd = float(CRS)
    shared = [1.0, cfd, -cfd, float(_f32(cfd * LF)), crd,
              float(_f32(-crd * LR)), -1.0]
    hdiag = {}
    dset = list(shared)
    for h32 in sorted(set(float(v) for v in hs32)):
        h = float(h32)
        vals = [h,
                float(_f32(h * FY_F_MAX / M_)), float(_f32(h * FY_R_MAX / M_)),
                float(_f32(h * LF * FY_F_MAX / IZ)),
                float(_f32(-h * LR * FY_R_MAX / IZ))]
        hdiag[h] = list(range(len(dset), len(dset) + len(vals)))
        dset.extend(vals)
    D_ONE, D_CF, D_NCF, D_CFLF, D_CR, D_NCRLR, D_NEG1 = 0, 1, 2, 3, 4, 5, 6
    ND = len(dset)

    wdiag_host = np.zeros((ND, P, P), dtype=np.float32)
    eye = np.eye(P, dtype=np.float32)
    for i, c in enumerate(dset):
        wdiag_host[i] = eye * _f32(c)

    nc = bacc.Bacc("TRN2", target_bir_lowering=False, debug=False)
    st_d = nc.declare_dram_parameter("state", [n_veh, 9], mybir.dt.float32,
                                     isOutput=False)
    ac_d = nc.declare_dram_parameter("action", [n_veh, 2], mybir.dt.float32,
                                     isOutput=False)
    wd_d = nc.declare_dram_parameter("wdiag", [ND, P, P], mybir.dt.float32,
                                     isOutput=False)
    out_d = nc.declare_dram_parameter("out", [n_veh, 9], mybir.dt.float32,
                                      isOutput=True)

    f32 = mybir.dt.float32
    f32r = mybir.dt.float32r

    RECIP = _REG["ANT_BIKE_RECIP_NR1"]
    PHISTEP = _REG["ANT_BIKE_PHI_STEP"]
    WRAPRN = _REG["ANT_BIKE_WRAP_RN"]
    DCLIP = _REG["ANT_BIKE_DCLIP"]
    RELUADD = _REG["ANT_BIKE_RELUADD"]

    with tile.TileContext(nc) as tc:
        with (
            tc.tile_pool(name="persist", bufs=1) as pp,
            tc.tile_pool(name="scratch", bufs=2) as sp,
            tc.tile_pool(name="psum", bufs=1, space="PSUM") as qq,
        ):
            # persistent SBUF
            big_in = pp.tile([P, FD * 9], f32)
            big_ac = pp.tile([P, FD * 2], f32)
            big_out = pp.tile([P, FD * 9], f32)
            wsb = pp.tile([P, ND * P], f32)
            wsr = pp.tile([P, ND * P], f32r)
            # beta lives in PSUM: beta_q += mm(-1,hr) + mm(1,w) per step --
            # PSUM adds are exact fp32 and only the f32r-rounded INCREMENTS
            # carry rounding (~1e-5 total). The X/Y matmuls read an f32r
            # SBUF mirror refreshed per step by ACT (PSUM reads are cheap
            # there). This keeps GpSimd nearly idle: DVE and GpSimd share
            # SBUF ports, and concurrent GpSimd traffic was measured to
            # inflate every DVE op 2-3x.
            beta_m = pp.tile([P, FD], f32r)
            # parity double-buffers: step k reads [k%2], writes [(k+1)%2];
            # lets updates run early in the queue while late readers still
            # see the old value.
            v_b = [pp.tile([P, FD], f32, name=f"v{i}") for i in range(2)]
            inv_b = [pp.tile([P, FD], f32, name=f"inv{i}") for i in range(2)]
            phi_b = [pp.tile([P, FD], f32, name=f"phi{i}") for i in range(2)]
            # delta: the clip can only bind on step 0 (afterwards |delta|
            # stays <= MAX_STEER and the recurrence is affine), so steps
            # k>=1 use delta_k = dref + dd with dd (f32r) decaying by QD.
            de0 = pp.tile([P, FD], f32)
            de1 = pp.tile([P, FD], f32)
            dd_t = pp.tile([P, FD], f32r)
            dref_m = pp.tile([P, FD], f32r)
            va_t = pp.tile([P, FD], f32)       # h*a (exact stt recurrence)
            haref2_c = pp.tile([P, FD], f32)   # (h^2/tau_a)*aref
            dref_c = pp.tile([P, FD], f32)
            qd_full = pp.tile([P, FD], f32)
            halfpi_b = pp.tile([P, 1], f32)
            nc.gpsimd.memset(halfpi_b[:], float(HALF_PI))
            # Pin the ACT table set: Silu exists only in silu_and_others,
            # which also holds Sin/Tanh/Abs/Copy -- one table load total.
            nc.scalar.activation(halfpi_b[:], halfpi_b[:], ACT.Silu)
            nc.gpsimd.memset(halfpi_b[:], float(HALF_PI))

            # PSUM: xy 1 bank, XY 2 (X in bank A, Y in bank B), S1 1;
            # r and psi are persistent PSUM accumulators (init hi/lo mms,
            # then two/one accumulating mms per step; V reads them mid-group)
            xy_q = qq.tile([P, 2 * FD], f32)
            XY_q = qq.tile([P, 4 * FD], f32)
            X_q = XY_q[:, 0:FD]
            Y_q = XY_q[:, 2 * FD:3 * FD]
            S1_q = qq.tile([P, FD], f32)
            r_q = qq.tile([P, FD], f32)
            psi_q = qq.tile([P, FD], f32)
            beta_q = qq.tile([P, FD], f32)

            def W(i):
                return wsr[:, bass.ts(i, P)]

            def mm(out_ap, didx, rhs_ap, start, stop):
                nc.tensor.matmul(out_ap, W(didx), rhs_ap,
                                 start=start, stop=stop)

            # ---------------- load + unpack ----------------
            nc.sync.dma_start(big_in[:], st_d[:].rearrange(
                "(p q) v -> p (q v)", p=P))
            nc.sync.dma_start(big_ac[:], ac_d[:].rearrange(
                "(p q) v -> p (q v)", p=P))
            nc.sync.dma_start(wsb[:].rearrange("p (d m) -> p d m", m=P),
                              wd_d[:].rearrange("d k m -> k d m"))
            nc.vector.tensor_copy(wsr[:], wsb[:])  # fp32 -> fp32r weights

            sv = big_in[:].rearrange("p (q v) -> p q v", v=9)
            av = big_ac[:].rearrange("p (q v) -> p q v", v=2)
            xy0 = sp.tile([P, 2 * FD], f32, tag="init0")
            psi0 = sp.tile([P, FD], f32, tag="init2")
            a0 = sp.tile([P, FD], f32, tag="init3")
            r0 = sp.tile([P, FD], f32, tag="init_r")
            beta0 = sp.tile([P, FD], f32, tag="init_beta")
            nc.vector.tensor_copy(xy0[:, 0:FD], sv[:, :, 0])
            nc.vector.tensor_copy(xy0[:, FD:2 * FD], sv[:, :, 1])
            nc.vector.tensor_copy(psi0[:], sv[:, :, 2])
            nc.scalar.copy(v_b[0][:], sv[:, :, 3])
            nc.scalar.copy(a0[:], sv[:, :, 4])
            nc.vector.tensor_copy(de0[:], sv[:, :, 5])
            nc.vector.tensor_copy(beta0[:], sv[:, :, 6])
            nc.vector.tensor_copy(r0[:], sv[:, :, 7])

            aref_c = sp.tile([P, FD], f32, tag="init_aref")
            nc.vector.tensor_scalar(aref_c[:], av[:, :, 0], float(MIN_ACC),
                                    float(MAX_ACC), alu.max, alu.min)
            nc.vector.tensor_scalar(dref_c[:], av[:, :, 1], float(-MS),
                                    float(MS), alu.max, alu.min)
            nc.scalar.copy(dref_m[:], dref_c[:])
            nc.gpsimd.memset(qd_full[:], float(_f32(1.0)
                                               - _f32(_f32(h_base)
                                                      / _f32(TAU_D))))
            # va = h*a ; haref2 = (h^2/tau_a)*aref
            nc.vector.tensor_scalar(va_t[:], a0[:], h_base, None, alu.mult)
            nc.vector.tensor_scalar(haref2_c[:], aref_c[:],
                                    float(_f32(_f32(_f32(h_base)
                                               * _f32(h_base))
                                               / _f32(TAU_A))),
                                    None, alu.mult)
            # exact reciprocal seed
            ve0 = sp.tile([P, FD], f32, tag="init4")
            nc.vector.tensor_scalar(ve0[:], v_b[0][:], float(VMIN), None,
                                    alu.max)
            nc.vector.reciprocal(inv_b[0][:], ve0[:])
            # wrapped phi = psi + beta
            pb0 = sp.tile([P, FD], f32, tag="init5")
            nc.vector.tensor_add(pb0[:], psi0[:], beta0[:])
            nc.vector._custom_dve(WRAPRN, out=phi_b[0][:], in0=pb0[:],
                                  s0=float(INV_2PI), s1=float(MAGIC),
                                  imm2=float(TWO_PI))

            # exact PSUM init via hi/lo split (fp32r rounds moving values)
            xy0_hi = sp.tile([P, 2 * FD], f32r, tag="init6")
            xy0_lo = sp.tile([P, 2 * FD], f32r, tag="init7")
            psi0_hi = sp.tile([P, FD], f32r, tag="init8")
            psi0_lo = sp.tile([P, FD], f32r, tag="init9")
            r0_hi = sp.tile([P, FD], f32r, tag="init10")
            r0_lo = sp.tile([P, FD], f32r, tag="init11")
            b0_hi = sp.tile([P, FD], f32r, tag="init12")
            b0_lo = sp.tile([P, FD], f32r, tag="init13")
            nc.scalar.copy(xy0_hi[:], xy0[:])
            nc.vector.tensor_tensor(xy0_lo[:], xy0[:], xy0_hi[:],
                                    alu.subtract)
            nc.scalar.copy(psi0_hi[:], psi0[:])
            nc.vector.tensor_tensor(psi0_lo[:], psi0[:], psi0_hi[:],
                                    alu.subtract)
            nc.scalar.copy(r0_hi[:], r0[:])
            nc.vector.tensor_tensor(r0_lo[:], r0[:], r0_hi[:], alu.subtract)
            nc.scalar.copy(b0_hi[:], beta0[:])
            nc.vector.tensor_tensor(b0_lo[:], beta0[:], b0_hi[:],
                                    alu.subtract)
            nc.scalar.copy(beta_m[:], beta0[:])
            laststop = n_steps == 0
            mm(xy_q[:], D_ONE, xy0_hi[:], start=True, stop=False)
            mm(xy_q[:], D_ONE, xy0_lo[:], start=False, stop=laststop)
            mm(psi_q[:], D_ONE, psi0_hi[:], start=True, stop=False)
            mm(psi_q[:], D_ONE, psi0_lo[:], start=False, stop=laststop)
            mm(r_q[:], D_ONE, r0_hi[:], start=True, stop=False)
            mm(r_q[:], D_ONE, r0_lo[:], start=False, stop=laststop)
            mm(beta_q[:], D_ONE, b0_hi[:], start=True, stop=False)
            mm(beta_q[:], D_ONE, b0_lo[:], start=False, stop=laststop)

            # ---------------- main loop ----------------
            # Software-pipelined: rv for step k is computed at the tail of
            # step k-1 (right after w, when r is fresh); the xy matmul for
            # step k-1's trig runs at the START of step k, filling the PE
            # gap and keeping it warm. Both carry tiles across iterations.
            rv_cur = sp.tile([P, FD], f32r, tag="rv")
            nc.vector.tensor_tensor(rv_cur[:], r_q[:], inv_b[0][:], alu.mult)
            vcvs_prev = None
            for k in range(n_steps):
                h = float(hs32[k])
                DH, DC1, DC2, DK1, DK2 = hdiag[h]
                QA = float(_f32(1.0) - _f32(_f32(h) / _f32(TAU_A)))
                QD = float(_f32(1.0) - _f32(_f32(h) / _f32(TAU_D)))
                CD = float(_f32(_f32(h) / _f32(TAU_D)))
                hh = float(_f32(h) / _f32(h_base))
                last = (k + 1 == n_steps)

                v_cur, v_nxt = v_b[k % 2], v_b[(k + 1) % 2]
                inv_cur, inv_nxt = inv_b[k % 2], inv_b[(k + 1) % 2]
                phi_cur, phi_nxt = phi_b[k % 2], phi_b[(k + 1) % 2]

                # scratch tiles for this step (double-buffered pool)
                hr = sp.tile([P, FD], f32r, tag="hr")
                trig = sp.tile([P, 2 * FD], f32, tag="trig")
                absphi = sp.tile([P, FD], f32, tag="absphi")
                TfTr = sp.tile([P, 2 * FD], f32r, tag="TfTr")
                Tf = TfTr[:, 0:FD]
                Tr = TfTr[:, FD:2 * FD]
                w_t = sp.tile([P, FD], f32r, tag="w")
                vcvs = sp.tile([P, 2 * FD], f32r, tag="vcvs")

                # ---- ACT head: hr = h*r (PSUM read, f32r out) ----
                nc.scalar.mul(hr[:], r_q[:], h)
                if k == 0:
                    # step 0 uses the raw (possibly out-of-range) delta
                    de_m = sp.tile([P, FD], f32r, tag="de_m")
                    nc.scalar.copy(de_m[:], de0[:])

                # ---- PE: previous step's xy accumulation first -- fills
                # the cold gap and warms the array for the X chain ----
                if vcvs_prev is not None:
                    mm(xy_q[:], DH_prev, vcvs_prev[:], start=False,
                       stop=False)

                # ---- PE: X, Y. X delta terms go first (constant operands,
                # run before rv lands); beta mms last (freshest dep) ----
                if k == 0:
                    mm(X_q, D_NCF, de_m[:], start=True, stop=False)
                else:
                    mm(X_q, D_NCF, dref_m[:], start=True, stop=False)
                    mm(X_q, D_NCF, dd_t[:], start=False, stop=False)
                mm(X_q, D_CFLF, rv_cur[:], start=False, stop=False)
                mm(Y_q, D_NCRLR, rv_cur[:], start=True, stop=False)
                mm(X_q, D_CF, beta_m[:], start=False, stop=True)
                mm(Y_q, D_CR, beta_m[:], start=False, stop=True)

                # ---- DVE fillers (run while PE/ACT chew on X/tanh) ----
                if k == 0:
                    nc.vector._custom_dve(DCLIP, out=de1[:], in0=de0[:],
                                          in1=dref_c[:], s0=QD,
                                          s1=float(-MS), imm2=CD)
                    if n_steps > 1:
                        # dd = delta_1 - dref (f32r; decays by QD onwards)
                        nc.vector.tensor_tensor(dd_t[:], de1[:], dref_c[:],
                                                alu.subtract)
                nc.vector._custom_dve(RELUADD, out=v_nxt[:], in0=v_cur[:],
                                      in1=va_t[:], s0=hh)
                if not last:
                    nc.vector._custom_dve(RECIP, out=inv_nxt[:],
                                          in0=v_nxt[:], in1=inv_cur[:],
                                          s0=float(VMIN), s1=2.0)
                # va' = QA*va + (h^2/tau)*aref (reads old va after RELUADD)
                nc.vector.scalar_tensor_tensor(va_t[:], va_t[:], QA,
                                               haref2_c[:], alu.mult, alu.add)

                # one tanh over [X | Y] (strided 2-bank PSUM read)
                nc.scalar.activation(
                    TfTr[:].rearrange("p (a b) -> p a b", a=2),
                    XY_q[:].rearrange("p (a b) -> p a b", a=4)[:, 0::2, :],
                    ACT.Tanh)

                # psi += h*r and beta -= h*r here: fills the PE idle window
                # under tanh so the S1 matmul issues warm
                mm(psi_q[:], D_ONE, hr[:], start=False, stop=last)
                mm(beta_q[:], D_NEG1, hr[:], start=False, stop=False)

                # S1 = c1*Tf + c2*Tr ; r += k1*Tf + k2*Tr (open accumulator)
                mm(S1_q[:], DC1, Tf, start=True, stop=False)
                mm(S1_q[:], DC2, Tr, start=False, stop=True)
                mm(r_q[:], DK1, Tf, start=False, stop=False)
                mm(r_q[:], DK2, Tr, start=False, stop=last)

                # ---- cycle tail: w (f32r) -> beta_q += w on PE ----
                nc.vector.tensor_tensor(w_t[:], S1_q[:], inv_cur[:],
                                        alu.mult)
                mm(beta_q[:], D_ONE, w_t[:], start=False, stop=last)
                # f32r beta mirror for next step's X/Y matmuls (ACT reads
                # PSUM cheaply; DVE ts pays ~740ns for the f32r convert)
                nc.scalar.copy(beta_m[:], beta_q[:])
                if not last:
                    # rv for the NEXT step: r_q now holds r_{k+1}
                    rv_nxt = sp.tile([P, FD], f32r, tag="rv")
                    nc.vector.tensor_tensor(rv_nxt[:], r_q[:], inv_nxt[:],
                                            alu.mult)
                    nc.vector._custom_dve(PHISTEP, out=phi_nxt[:],
                                          in0=phi_cur[:],
                                          in1=w_t[:], s0=float(PI_F),
                                          imm2=float(TWO_PI))
                    if k >= 1:
                        # decay dd for the next step (GpSimd tt; ACT is
                        # queue-limited and a DVE ts pays the f32r penalty)
                        nc.gpsimd.tensor_tensor(dd_t[:], dd_t[:],
                                                qd_full[:], alu.mult)

                # ---- trig of phi_k for THIS step's xy increment (consumed
                # by the xy matmul at the start of step k+1; these read the
                # parity buffer so queue position is free) ----
                nc.scalar.activation(trig[:, FD:2 * FD], phi_cur[:], ACT.Sin)
                nc.scalar.activation(absphi[:], phi_cur[:], ACT.Abs)
                nc.scalar.activation(trig[:, 0:FD], absphi[:], ACT.Sin,
                                     bias=halfpi_b[:], scale=-1.0)
                nc.gpsimd.tensor_tensor(vcvs[:, FD:2 * FD],
                                        trig[:, FD:2 * FD],
                                        v_cur[:], alu.mult)
                nc.vector.tensor_tensor(vcvs[:, 0:FD], trig[:, 0:FD],
                                        v_cur[:], alu.mult)

                if not last:
                    rv_cur = rv_nxt
                vcvs_prev = vcvs
                DH_prev = DH

            # final step's xy accumulation
            if n_steps:
                mm(xy_q[:], DH_prev, vcvs_prev[:], start=False, stop=True)

            # ---------------- finalize ----------------
            pf = n_steps % 2
            ov = big_out[:].rearrange("p (q v) -> p q v", v=9)
            nc.vector.tensor_copy(ov[:, :, 0], xy_q[:, 0:FD])
            nc.vector.tensor_copy(ov[:, :, 1], xy_q[:, FD:2 * FD])
            nc.scalar.copy(ov[:, :, 2], psi_q[:])
            nc.vector.tensor_copy(ov[:, :, 3], v_b[pf][:])
            inv_hb = float(_f32(1.0) / _f32(h_base))
            nc.vector.tensor_scalar(ov[:, :, 4], va_t[:], inv_hb, None,
                                    alu.mult)
            if n_steps == 0:
                nc.vector.tensor_copy(ov[:, :, 5], de0[:])
            elif n_steps == 1:
                nc.vector.tensor_copy(ov[:, :, 5], de1[:])
            else:
                # delta_n = dref + QD_last * dd (one final decay)
                QDL = float(_f32(1.0)
                            - _f32(_f32(hs32[-1]) / _f32(TAU_D)))
                nc.vector.scalar_tensor_tensor(ov[:, :, 5], dd_t[:], QDL,
                                               dref_c[:], alu.mult, alu.add)
            nc.scalar.copy(ov[:, :, 6], beta_q[:])
            nc.vector.tensor_copy(ov[:, :, 7], r_q[:])
            nc.scalar.copy(ov[:, :, 8], dref_c[:])
            nc.sync.dma_start(out_d[:].rearrange("(p q) v -> p (q v)", p=P),
                              big_out[:])

    nc.compile()
    return nc, wdiag_host


_BUILD_CACHE = {}


def _get_built(dt_total, n_veh=B_CORE):
    hs = tuple(_step_hs(float(dt_total)))
    key = (hs, n_veh)
    if key not in _BUILD_CACHE:
        _BUILD_CACHE[key] = build_kernel(list(hs), n_veh)
    return _BUILD_CACHE[key]


def kernel(state, action, dt):
    state = np.ascontiguousarray(np.asarray(state, dtype=np.float32))
    action = np.ascontiguousarray(np.asarray(action, dtype=np.float32))
    assert state.shape == (B_TOTAL, 9) and action.shape == (B_TOTAL, 2)

    nc, wdiag = _get_built(float(dt))

    from concourse.bass_utils import run_bass_kernel_spmd

    st_sh = np.split(state, N_CORES, axis=0)
    ac_sh = np.split(action, N_CORES, axis=0)
    in_maps = [
        {"state": np.ascontiguousarray(st_sh[i]),
         "action": np.ascontiguousarray(ac_sh[i]),
         "wdiag": wdiag}
        for i in range(N_CORES)
    ]
    res = run_bass_kernel_spmd(nc, in_maps, core_ids=list(range(N_CORES)))
    out = np.concatenate([r["out"] for r in res.results], axis=0)
    return out.astype(np.float32)


if __name__ == "__main__":
    rng = np.random.default_rng(0)
    s = rng.standard_normal((B_TOTAL, 9), dtype=np.float32)
    a = rng.standard_normal((B_TOTAL, 2), dtype=np.float32)
    o = kernel(s, a, 1)
    print("out", o.shape, o.dtype, np.isfinite(o).all())
